# revision 1
# baseline (speedup 1.0000x reference)
"""InterleavedHeadAttention Trainium2 kernel.

Sharding (8 cores): core c handles batch b = c//4 and 4 output heads
[4*(c%4), 4*(c%4)+4).  The alpha head-mixing einsum is folded into the
QKV projection weights on the host, so each core's projections only
produce its own heads' (h, p, d) slices.  The pseudo-head merge uses
(p, n) flat ordering internally (attention is permutation invariant;
the token-causal mask depends only on n), which makes every layout a
direct view of a matmul output.  collapse and the 1/den softmax
normalization are applied on-device; Wo is folded with collapse and
applied per-head, each core emitting a partial (S, HID) f32 output
that the host sums (+bo).
"""
import numpy as np
import ml_dtypes

import concourse.bacc as bacc
import concourse.bass as bass
import concourse.tile as tile
import concourse.mybir as mybir
from concourse.bass_utils import run_bass_kernel_spmd

B, S, HID, H, P = 2, 1024, 1024, 16, 2
D = HID // H          # 64
HL = 4                # heads per core
G = HL * P            # (h,p) groups per core = 8
HPD = HL * P * D      # 512 projection rows per core
BF = mybir.dt.bfloat16
F32 = mybir.dt.float32
NCORES = 8

_compiled = None


def _build():
    nc = bacc.Bacc()
    xT = nc.dram_tensor("xT", (HID, S), BF, kind="ExternalInput")
    wq = nc.dram_tensor("wq", (HID, HPD), BF, kind="ExternalInput")
    wk = nc.dram_tensor("wk", (HID, HPD), BF, kind="ExternalInput")
    wv = nc.dram_tensor("wv", (HID, HPD), BF, kind="ExternalInput")
    bq = nc.dram_tensor("bq", (1, HPD), BF, kind="ExternalInput")
    bk = nc.dram_tensor("bk", (1, HPD), BF, kind="ExternalInput")
    bv = nc.dram_tensor("bv", (1, HPD), BF, kind="ExternalInput")
    wo = nc.dram_tensor("wo", (HL, P * D, HID), BF, kind="ExternalInput")
    tri = nc.dram_tensor("tri", (128, 128), BF, kind="ExternalInput")
    out = nc.dram_tensor("o", (S, HID), F32, kind="ExternalOutput")

    KT = HID // 128   # 8 k tiles over hidden
    NT = S // 512     # 2 n windows per p
    with tile.TileContext(nc) as tc:
        with tc.tile_pool(name="persist", bufs=1) as pp, \
             tc.tile_pool(name="ppool", bufs=4) as ppl, \
             tc.tile_pool(name="small", bufs=4) as sml, \
             tc.tile_pool(name="osb", bufs=3) as osb, \
             tc.tile_pool(name="ps", bufs=3, space=bass.MemorySpace.PSUM) as ps, \
             tc.tile_pool(name="psbc", bufs=1, space=bass.MemorySpace.PSUM) as psbc, \
             tc.tile_pool(name="psav", bufs=4, space=bass.MemorySpace.PSUM) as psav:

            ones = pp.tile([1, 512], BF, tag="ones", name="ones")
            nc.gpsimd.memset(ones[:], 1.0)
            tri_sb = pp.tile([128, 128], BF, tag="tri", name="tri")
            nc.gpsimd.dma_start(tri_sb[:], tri[:])

            xt_sb = [pp.tile([128, S], BF, tag=f"xt{k}", name=f"xt{k}") for k in range(KT)]
            for k in range(KT):
                nc.gpsimd.dma_start(xt_sb[k][:], xT[k * 128:(k + 1) * 128, :])

            w_sb = {}
            for nm, dram in (("q", wq), ("k", wk), ("v", wv)):
                w_sb[nm] = [pp.tile([128, HPD], BF, tag=f"w{nm}{k}", name=f"w{nm}{k}") for k in range(KT)]
                for k in range(KT):
                    nc.gpsimd.dma_start(w_sb[nm][k][:], dram[k * 128:(k + 1) * 128, :])
            b_sb = {}
            for nm, dram in (("q", bq), ("k", bk), ("v", bv)):
                b_sb[nm] = pp.tile([1, HPD], BF, tag=f"b{nm}", name=f"b{nm}")
                nc.gpsimd.dma_start(b_sb[nm][:], dram[:])
            woe_sb = [pp.tile([128, HID], BF, tag=f"woe{h}", name=f"woe{h}") for h in range(HL)]
            for h in range(HL):
                nc.gpsimd.dma_start(woe_sb[h][:], wo[h])

            # ---- Q/K transposed projections: out (hpd=512, n=1024) ----
            qt_sb = [pp.tile([128, S], BF, tag=f"qt{h}", name=f"qt{h}") for h in range(HL)]
            kt_sb = [pp.tile([128, S], BF, tag=f"kt{h}", name=f"kt{h}") for h in range(HL)]
            kt2_sb = [pp.tile([128, S], BF, tag=f"kt2{h}", name=f"kt2{h}") for h in range(HL)]
            for nm in ("q", "k"):
                for mt in range(HL):          # 128 hpd rows = head mt
                    for nt in range(NT):      # 512 seq cols
                        acc = ps.tile([128, 512], F32, tag="mm", name="mm")
                        for k in range(KT):
                            nc.tensor.matmul(
                                acc[:], w_sb[nm][k][:, mt * 128:(mt + 1) * 128],
                                xt_sb[k][:, nt * 512:(nt + 1) * 512],
                                start=(k == 0), stop=False)
                        nc.tensor.matmul(
                            acc[:], b_sb[nm][:, mt * 128:(mt + 1) * 128],
                            ones[:], start=False, stop=True)
                        dst = qt_sb[mt] if nm == "q" else kt_sb[mt]
                        sl = slice(nt * 512, (nt + 1) * 512)
                        nc.vector.tensor_copy(dst[:, sl], acc[:])
                        if nm == "k":
                            nc.vector.tensor_copy(kt2_sb[mt][0:64, sl], acc[64:128, :])
                            nc.vector.tensor_copy(kt2_sb[mt][64:128, sl], acc[0:64, :])

            # ---- V projection: out (n=1024, hpd=512) -> vaug (128, 8*65) ----
            vaug = [pp.tile([128, G * 65], BF, tag=f"va{j}", name=f"va{j}") for j in range(S // 128)]
            for jt in range(S // 128):
                v3 = vaug[jt].rearrange("p (g e) -> p g e", e=65)
                nc.gpsimd.memset(v3[:, :, 64:65], 1.0)
                acc = ps.tile([128, 512], F32, tag="mm", name="mm")
                for k in range(KT):
                    nc.tensor.matmul(
                        acc[:], xt_sb[k][:, jt * 128:(jt + 1) * 128],
                        w_sb["v"][k][:], start=(k == 0), stop=False)
                nc.tensor.matmul(acc[:], ones[:, 0:128], b_sb["v"][:],
                                 start=False, stop=True)
                nc.vector.tensor_copy(
                    v3[:, :, 0:64], acc[:].rearrange("p (g e) -> p g e", e=64))

            # ---- attention per head; (p,n) flat ordering ----
            ot2 = [pp.tile([128, S], BF, tag=f"ot2{h}", name=f"ot2{h}") for h in range(HL)]
            for h in range(HL):
                for In in range(NT):
                    avp = [psav.tile([65, 512], F32, tag="av", name="av") for _ in range(2)]
                    for Jn in range(4 * In + 4):
                        FF = 128 * (Jn - 4 * In)
                        part = FF >= 0
                        c0 = FF if part else 0
                        for pk in range(2):
                            # scores packed: rows 0-63 -> pq=0, rows 64-127 -> pq=1
                            lhsA = (kt_sb[h] if pk == 0 else kt2_sb[h])
                            lhsB = (kt2_sb[h] if pk == 0 else kt_sb[h])
                            jsl = slice(Jn * 128, (Jn + 1) * 128)
                            isl = slice(In * 512 + c0, (In + 1) * 512)
                            sp = [ps.tile([128, 512], F32, tag="mm", name="mm") for _ in range(2)]
                            nc.tensor.matmul(
                                sp[0][:, c0:512], lhsA[0:64, jsl],
                                qt_sb[h][0:64, isl], start=True, stop=True)
                            nc.tensor.matmul(
                                sp[1][:, c0:512], lhsB[64:128, jsl],
                                qt_sb[h][64:128, isl], start=True, stop=True)
                            for pq in range(2):
                                pt = ppl.tile([128, 512], BF, tag="p", name="p")
                                nc.scalar.activation(
                                    pt[:, c0:512], sp[pq][:, c0:512],
                                    mybir.ActivationFunctionType.Exp, scale=0.125)
                                if part:
                                    nc.vector.tensor_mul(
                                        pt[:, c0:c0 + 128], pt[:, c0:c0 + 128],
                                        tri_sb[:])
                                g = h * 2 + pk
                                nc.tensor.matmul(
                                    avp[pq][:, c0:512],
                                    vaug[Jn][:, g * 65:g * 65 + 65],
                                    pt[:, c0:512],
                                    start=(Jn == 0 and pk == 0),
                                    stop=(Jn == 4 * In + 3 and pk == 1))
                    for pq in range(2):
                        recip = sml.tile([1, 512], BF, tag="recip", name="recip")
                        with nc.allow_low_precision(reason="softmax recip bf16"):
                            nc.vector.reciprocal(recip[:], avp[pq][64:65, :])
                        bcp = psbc.tile([64, 512], F32, tag="bc", name="bc")
                        nc.tensor.matmul(bcp[:], ones[:, 0:64], recip[:],
                                         start=True, stop=True)
                        bcs = sml.tile([64, 512], F32, tag="bcs", name="bcs")
                        nc.vector.tensor_copy(bcs[:], bcp[:])
                        nc.vector.tensor_mul(
                            ot2[h][pq * 64:(pq + 1) * 64, In * 512:(In + 1) * 512],
                            avp[pq][0:64, :], bcs[:])

            # ---- output projection: partial (S, HID) f32 ----
            for mt in range(S // 128):
                for jt in range(HID // 512):
                    op = ps.tile([128, 512], F32, tag="mm", name="mm")
                    for h in range(HL):
                        nc.tensor.matmul(
                            op[:], ot2[h][:, mt * 128:(mt + 1) * 128],
                            woe_sb[h][:, jt * 512:(jt + 1) * 512],
                            start=(h == 0), stop=(h == HL - 1))
                    ob = osb.tile([128, 512], F32, tag="ob", name="ob")
                    nc.vector.tensor_copy(ob[:], op[:])
                    nc.gpsimd.dma_start(
                        out[mt * 128:(mt + 1) * 128, jt * 512:(jt + 1) * 512], ob[:])
    nc.compile()
    return nc


def _prep(inputs):
    bf = ml_dtypes.bfloat16
    hs = np.asarray(inputs["hidden_states"], np.float32)
    maps = []
    tri = np.triu(np.ones((128, 128), np.float32)).astype(bf)  # tri[r,c]=1 iff c>=r
    eff = {}
    for nm in ("q", "k", "v"):
        W = np.asarray(inputs[f"W{nm}"], np.float32)
        bb = np.asarray(inputs[f"b{nm}"], np.float32)
        al = np.asarray(inputs[f"alpha_{nm}"], np.float32)
        We = np.einsum("mhp,mdc->hpdc", al, W.reshape(H, D, HID))
        be = np.einsum("mhp,md->hpd", al, bb.reshape(H, D))
        eff[nm] = (We, be)
    Wo = np.asarray(inputs["Wo"], np.float32)
    col = np.asarray(inputs["collapse"], np.float32)
    Woe = np.einsum("hp,jhd->hpdj", col, Wo.reshape(HID, H, D))  # (H,P,D,HID)
    for c in range(NCORES):
        b, g = c // 4, c % 4
        hs_sl = slice(g * HL, (g + 1) * HL)
        m = {"xT": np.ascontiguousarray(hs[b].T).astype(bf),
             "tri": tri}
        for nm in ("q", "k", "v"):
            We, be = eff[nm]
            Wslice = We[hs_sl].reshape(HPD, HID)      # (hpd, c)
            m[f"w{nm}"] = np.ascontiguousarray(Wslice.T).astype(bf)
            m[f"b{nm}"] = be[hs_sl].reshape(1, HPD).astype(bf)
        m["wo"] = Woe[hs_sl].reshape(HL, P * D, HID).astype(bf)
        maps.append(m)
    return maps


def kernel(**inputs):
    global _compiled
    if _compiled is None:
        _compiled = _build()
    maps = _prep(inputs)
    res = run_bass_kernel_spmd(_compiled, maps, core_ids=list(range(NCORES)))
    bo = np.asarray(inputs["bo"], np.float32)
    out = np.zeros((B, S, HID), np.float32)
    for c in range(NCORES):
        out[c // 4] += res.results[c]["o"]
    out += bo
    return out



# revision 2
# speedup vs baseline: 28.9447x; 28.9447x over previous
"""InterleavedHeadAttention Trainium2 kernel.

Sharding (8 cores): core c handles batch b = c//4 and 4 output heads
[4*(c%4), 4*(c%4)+4).  The alpha head-mixing einsum is folded into the
QKV projection weights on the host, so each core's projections only
produce its own heads' (h, p, d) slices.  The pseudo-head merge uses
(p, n) flat ordering internally (attention is permutation invariant;
the token-causal mask depends only on n), which makes every layout a
direct view of a matmul output.  collapse and the 1/den softmax
normalization are applied on-device; Wo is folded with collapse and
applied per-head, each core emitting a partial (S, HID) bf16 output
that the host sums in f32 (+bo).

All per-core inputs are packed into a single 1-D bf16 "blob" tensor:
per-exec dispatch overhead in the PJRT/axon path scales with the number
of bound buffers, so 17 inputs -> 1 input is a large wall-clock win.
"""
import numpy as np
import ml_dtypes

import concourse.bacc as bacc
import concourse.bass as bass
import concourse.tile as tile
import concourse.mybir as mybir
from concourse.bass_utils import run_bass_kernel_spmd

B, S, HID, H, P = 2, 1024, 1024, 16, 2
D = HID // H          # 64
HL = 4                # heads per core
G = HL * P            # (h,p) groups per core = 8
HPD = HL * P * D      # 512 projection rows per core
BF = mybir.dt.bfloat16
F32 = mybir.dt.float32
NCORES = 8

# blob layout (bf16 element offsets)
OFF_XT = 0                          # (HID, S) = (1024, 1024)
OFF_WQ = OFF_XT + HID * S           # (HID, HPD)
OFF_WK = OFF_WQ + HID * HPD
OFF_WV = OFF_WK + HID * HPD
OFF_BQ = OFF_WV + HID * HPD         # (512,) each
OFF_BK = OFF_BQ + HPD
OFF_BV = OFF_BK + HPD
OFF_WO = OFF_BV + HPD               # (HL, P*D, HID)
OFF_TRI = OFF_WO + HL * P * D * HID  # (128, 128)
TOT = OFF_TRI + 128 * 128

_compiled = None


def _build():
    nc = bacc.Bacc()
    blob = nc.dram_tensor("blob", (TOT,), BF, kind="ExternalInput")
    out = nc.dram_tensor("o", (S, HID), BF, kind="ExternalOutput")

    def bview(off, p, e):
        return blob[off:off + p * e].rearrange("(p e) -> p e", e=e)

    KT = HID // 128   # 8 k tiles over hidden
    NT = S // 512     # 2 n windows per p
    with tile.TileContext(nc) as tc:
        with tc.tile_pool(name="persist", bufs=1) as pp, \
             tc.tile_pool(name="ppool", bufs=4) as ppl, \
             tc.tile_pool(name="small", bufs=4) as sml, \
             tc.tile_pool(name="osb", bufs=3) as osb, \
             tc.tile_pool(name="ps", bufs=3, space=bass.MemorySpace.PSUM) as ps, \
             tc.tile_pool(name="psbc", bufs=1, space=bass.MemorySpace.PSUM) as psbc, \
             tc.tile_pool(name="psav", bufs=4, space=bass.MemorySpace.PSUM) as psav:

            ones = pp.tile([1, 512], BF, tag="ones", name="ones")
            nc.gpsimd.memset(ones[:], 1.0)
            tri_sb = pp.tile([128, 128], BF, tag="tri", name="tri")
            nc.sync.dma_start(tri_sb[:], bview(OFF_TRI, 128, 128))

            xt_sb = [pp.tile([128, S], BF, tag=f"xt{k}", name=f"xt{k}") for k in range(KT)]
            for k in range(KT):
                nc.sync.dma_start(xt_sb[k][:], bview(OFF_XT + k * 128 * S, 128, S))

            w_sb = {}
            for nm, off in (("q", OFF_WQ), ("k", OFF_WK), ("v", OFF_WV)):
                w_sb[nm] = [pp.tile([128, HPD], BF, tag=f"w{nm}{k}", name=f"w{nm}{k}") for k in range(KT)]
                for k in range(KT):
                    nc.sync.dma_start(w_sb[nm][k][:],
                                      bview(off + k * 128 * HPD, 128, HPD))
            b_sb = {}
            for nm, off in (("q", OFF_BQ), ("k", OFF_BK), ("v", OFF_BV)):
                b_sb[nm] = pp.tile([1, HPD], BF, tag=f"b{nm}", name=f"b{nm}")
                nc.sync.dma_start(b_sb[nm][:], bview(off, 1, HPD))
            woe_sb = [pp.tile([128, HID], BF, tag=f"woe{h}", name=f"woe{h}") for h in range(HL)]
            for h in range(HL):
                nc.sync.dma_start(woe_sb[h][:],
                                  bview(OFF_WO + h * P * D * HID, 128, HID))

            # ---- Q/K transposed projections: out (hpd=512, n=1024) ----
            qt_sb = [pp.tile([128, S], BF, tag=f"qt{h}", name=f"qt{h}") for h in range(HL)]
            kt_sb = [pp.tile([128, S], BF, tag=f"kt{h}", name=f"kt{h}") for h in range(HL)]
            kt2_sb = [pp.tile([128, S], BF, tag=f"kt2{h}", name=f"kt2{h}") for h in range(HL)]
            for nm in ("q", "k"):
                for mt in range(HL):          # 128 hpd rows = head mt
                    for nt in range(NT):      # 512 seq cols
                        acc = ps.tile([128, 512], F32, tag="mm", name="mm")
                        for k in range(KT):
                            nc.tensor.matmul(
                                acc[:], w_sb[nm][k][:, mt * 128:(mt + 1) * 128],
                                xt_sb[k][:, nt * 512:(nt + 1) * 512],
                                start=(k == 0), stop=False)
                        nc.tensor.matmul(
                            acc[:], b_sb[nm][:, mt * 128:(mt + 1) * 128],
                            ones[:], start=False, stop=True)
                        dst = qt_sb[mt] if nm == "q" else kt_sb[mt]
                        sl = slice(nt * 512, (nt + 1) * 512)
                        nc.vector.tensor_copy(dst[:, sl], acc[:])
                        if nm == "k":
                            nc.vector.tensor_copy(kt2_sb[mt][0:64, sl], acc[64:128, :])
                            nc.vector.tensor_copy(kt2_sb[mt][64:128, sl], acc[0:64, :])

            # ---- V projection: out (n=1024, hpd=512) -> vaug (128, 8*65) ----
            vaug = [pp.tile([128, G * 65], BF, tag=f"va{j}", name=f"va{j}") for j in range(S // 128)]
            for jt in range(S // 128):
                v3 = vaug[jt].rearrange("p (g e) -> p g e", e=65)
                nc.gpsimd.memset(v3[:, :, 64:65], 1.0)
                acc = ps.tile([128, 512], F32, tag="mm", name="mm")
                for k in range(KT):
                    nc.tensor.matmul(
                        acc[:], xt_sb[k][:, jt * 128:(jt + 1) * 128],
                        w_sb["v"][k][:], start=(k == 0), stop=False)
                nc.tensor.matmul(acc[:], ones[:, 0:128], b_sb["v"][:],
                                 start=False, stop=True)
                nc.vector.tensor_copy(
                    v3[:, :, 0:64], acc[:].rearrange("p (g e) -> p g e", e=64))

            # ---- attention per head; (p,n) flat ordering ----
            ot2 = [pp.tile([128, S], BF, tag=f"ot2{h}", name=f"ot2{h}") for h in range(HL)]
            for h in range(HL):
                for In in range(NT):
                    avp = [psav.tile([65, 512], F32, tag="av", name="av") for _ in range(2)]
                    for Jn in range(4 * In + 4):
                        FF = 128 * (Jn - 4 * In)
                        part = FF >= 0
                        c0 = FF if part else 0
                        for pk in range(2):
                            # scores packed: rows 0-63 -> pq=0, rows 64-127 -> pq=1
                            lhsA = (kt_sb[h] if pk == 0 else kt2_sb[h])
                            lhsB = (kt2_sb[h] if pk == 0 else kt_sb[h])
                            jsl = slice(Jn * 128, (Jn + 1) * 128)
                            isl = slice(In * 512 + c0, (In + 1) * 512)
                            sp = [ps.tile([128, 512], F32, tag="mm", name="mm") for _ in range(2)]
                            nc.tensor.matmul(
                                sp[0][:, c0:512], lhsA[0:64, jsl],
                                qt_sb[h][0:64, isl], start=True, stop=True)
                            nc.tensor.matmul(
                                sp[1][:, c0:512], lhsB[64:128, jsl],
                                qt_sb[h][64:128, isl], start=True, stop=True)
                            for pq in range(2):
                                pt = ppl.tile([128, 512], BF, tag="p", name="p")
                                nc.scalar.activation(
                                    pt[:, c0:512], sp[pq][:, c0:512],
                                    mybir.ActivationFunctionType.Exp, scale=0.125)
                                if part:
                                    nc.vector.tensor_mul(
                                        pt[:, c0:c0 + 128], pt[:, c0:c0 + 128],
                                        tri_sb[:])
                                g = h * 2 + pk
                                nc.tensor.matmul(
                                    avp[pq][:, c0:512],
                                    vaug[Jn][:, g * 65:g * 65 + 65],
                                    pt[:, c0:512],
                                    start=(Jn == 0 and pk == 0),
                                    stop=(Jn == 4 * In + 3 and pk == 1))
                    for pq in range(2):
                        recip = sml.tile([1, 512], BF, tag="recip", name="recip")
                        with nc.allow_low_precision(reason="softmax recip bf16"):
                            nc.vector.reciprocal(recip[:], avp[pq][64:65, :])
                        bcp = psbc.tile([64, 512], F32, tag="bc", name="bc")
                        nc.tensor.matmul(bcp[:], ones[:, 0:64], recip[:],
                                         start=True, stop=True)
                        bcs = sml.tile([64, 512], F32, tag="bcs", name="bcs")
                        nc.vector.tensor_copy(bcs[:], bcp[:])
                        nc.vector.tensor_mul(
                            ot2[h][pq * 64:(pq + 1) * 64, In * 512:(In + 1) * 512],
                            avp[pq][0:64, :], bcs[:])

            # ---- output projection: partial (S, HID) bf16 ----
            for mt in range(S // 128):
                for jt in range(HID // 512):
                    op = ps.tile([128, 512], F32, tag="mm", name="mm")
                    for h in range(HL):
                        nc.tensor.matmul(
                            op[:], ot2[h][:, mt * 128:(mt + 1) * 128],
                            woe_sb[h][:, jt * 512:(jt + 1) * 512],
                            start=(h == 0), stop=(h == HL - 1))
                    ob = osb.tile([128, 512], BF, tag="ob", name="ob")
                    nc.vector.tensor_copy(ob[:], op[:])
                    nc.sync.dma_start(
                        out[mt * 128:(mt + 1) * 128, jt * 512:(jt + 1) * 512], ob[:])
    nc.compile()
    return nc


def _prep(inputs):
    bf = ml_dtypes.bfloat16
    hs = np.asarray(inputs["hidden_states"], np.float32)
    maps = []
    tri = np.triu(np.ones((128, 128), np.float32)).astype(bf)  # tri[r,c]=1 iff c>=r
    eff = {}
    for nm in ("q", "k", "v"):
        W = np.asarray(inputs[f"W{nm}"], np.float32)
        bb = np.asarray(inputs[f"b{nm}"], np.float32)
        al = np.asarray(inputs[f"alpha_{nm}"], np.float32)
        We = np.einsum("mhp,mdc->hpdc", al, W.reshape(H, D, HID))
        be = np.einsum("mhp,md->hpd", al, bb.reshape(H, D))
        eff[nm] = (We, be)
    Wo = np.asarray(inputs["Wo"], np.float32)
    col = np.asarray(inputs["collapse"], np.float32)
    Woe = np.einsum("hp,jhd->hpdj", col, Wo.reshape(HID, H, D))  # (H,P,D,HID)
    for c in range(NCORES):
        b, g = c // 4, c % 4
        hs_sl = slice(g * HL, (g + 1) * HL)
        parts = [np.ascontiguousarray(hs[b].T).astype(bf).reshape(-1)]
        for nm in ("q", "k", "v"):
            We, _ = eff[nm]
            Wslice = We[hs_sl].reshape(HPD, HID)      # (hpd, c)
            parts.append(np.ascontiguousarray(Wslice.T).astype(bf).reshape(-1))
        for nm in ("q", "k", "v"):
            _, be = eff[nm]
            parts.append(be[hs_sl].reshape(-1).astype(bf))
        parts.append(Woe[hs_sl].reshape(-1).astype(bf))
        parts.append(tri.reshape(-1))
        blob = np.concatenate(parts)
        assert blob.shape[0] == TOT, blob.shape
        maps.append({"blob": blob})
    return maps


def kernel(**inputs):
    global _compiled
    if _compiled is None:
        _compiled = _build()
    maps = _prep(inputs)
    res = run_bass_kernel_spmd(_compiled, maps, core_ids=list(range(NCORES)))
    bo = np.asarray(inputs["bo"], np.float32)
    out = np.zeros((B, S, HID), np.float32)
    for c in range(NCORES):
        out[c // 4] += res.results[c]["o"].astype(np.float32)
    out += bo
    return out
